# revision 10
# baseline (speedup 1.0000x reference)
"""Bahdanau attention on 8 trn2 NeuronCores, data-parallel over batch.

Per batch item (S=4096, H=256):
  k^T[h,s]  = sum_d W2[d,h] * encT[d,s]            (PE, bf16, fp32 accum)
  energyT   = tanh(k^T + (hidden@W1 + b1 + b2)[h]) (ACT, bias-folded)
  scores    = energyT^T @ V       -> [s=128p, 32]  (PE, energy as stationary)
  softmax   = exp + row-sum accum + ones-matmul partition sum (fp32)
  context   = sum_s attn[s] * enc[s,:]             (PE, attn cols as stationary)

Host precomputes q-bias (hidden@W1_w + W1_b + W2_b), pre-transposes
encoder_outputs, and casts the big operands to bf16. V_b cancels in softmax.
"""

import sys

import numpy as np

try:
    import concourse.bass as bass
except ImportError:
    sys.path.insert(0, "/opt/trn_rl_repo")
    import concourse.bass as bass

import concourse.tile as tile
from concourse import bacc, mybir
from concourse.bass_utils import run_bass_kernel_spmd

B, S, H = 32, 4096, 256
NCORES = 8
BL = B // NCORES          # batch items per core
P = 128                   # partitions
NS = S // P               # 32 s-blocks of 128
NC5 = S // 512            # 8 s-chunks of 512

F32 = mybir.dt.float32
BF16 = mybir.dt.bfloat16

_CACHE = {}
TRACE = False           # set by test harness to capture an NTFF profile
LAST_EXEC_NS = None
LAST_TRACE_DIR = None


def _build():
    nc = bacc.Bacc("TRN2", target_bir_lowering=False, debug=False,
                   num_devices=NCORES)
    enc_nat = nc.declare_dram_parameter("enc_nat", [BL, P, NS * H], BF16, isOutput=False)
    encT = nc.declare_dram_parameter("encT", [BL, 2, P, S], BF16, isOutput=False)
    w2 = nc.declare_dram_parameter("w2", [H, H], BF16, isOutput=False)
    vw = nc.declare_dram_parameter("vw", [H, 1], BF16, isOutput=False)
    qbt = nc.declare_dram_parameter("qbt", [H, BL], F32, isOutput=False)
    ident = nc.declare_dram_parameter("ident", [P, P], F32, isOutput=False)
    attn_out = nc.declare_dram_parameter("attn_out", [BL, NS, P], F32, isOutput=True)
    ctx_out = nc.declare_dram_parameter("ctx_out", [BL, H], F32, isOutput=True)

    with tile.TileContext(nc) as tc:
        with (
            tc.tile_pool(name="singles", bufs=1) as singles,
            tc.tile_pool(name="enc", bufs=2) as encp,
            tc.tile_pool(name="energy", bufs=2) as enp,
            tc.tile_pool(name="sm", bufs=2) as smp,
            tc.tile_pool(name="psk", bufs=2, space="PSUM") as psk,
            tc.tile_pool(name="pss", bufs=1, space="PSUM") as pss,
            tc.tile_pool(name="psm", bufs=2, space="PSUM") as psm,
        ):
            # --- prologue: small operands ---
            w2_sb = [[singles.tile([P, P], BF16, tag=f"w2_{i}{j}", name=f"w2_{i}{j}") for j in range(2)]
                     for i in range(2)]
            for i in range(2):
                for j in range(2):
                    nc.gpsimd.dma_start(w2_sb[i][j][:], w2[i * P:(i + 1) * P, j * P:(j + 1) * P])
            v_sb = [singles.tile([P, 1], BF16, tag=f"v_{j}", name=f"v_{j}") for j in range(2)]
            for j in range(2):
                nc.gpsimd.dma_start(v_sb[j][:], vw[j * P:(j + 1) * P, :])
            qbt_sb = [singles.tile([P, BL], F32, tag=f"qbt_{j}", name=f"qbt_{j}") for j in range(2)]
            for j in range(2):
                nc.gpsimd.dma_start(qbt_sb[j][:], qbt[j * P:(j + 1) * P, :])
            ident_sb = singles.tile([P, P], F32, tag="ident")
            nc.gpsimd.dma_start(ident_sb[:], ident[:])
            ones_col = singles.tile([P, 1], F32, tag="ones_col")
            nc.vector.memset(ones_col[:], 1.0)
            ones_row = singles.tile([1, P], F32, tag="ones_row")
            nc.vector.memset(ones_row[:], 1.0)

            def pass1(b):
                # encT first (k-projection consumes it), in 512K slices
                encT_sb = [encp.tile([P, S], BF16, tag=f"encT_{i}", name=f"encT_{i}_{b}") for i in range(2)]
                for i in range(2):
                    for h in range(2):
                        sl = slice(h * 2048, (h + 1) * 2048)
                        nc.sync.dma_start(encT_sb[i][:, sl], encT[b, i][:, sl])

                # k^T then tanh(..+q) -> energyT (bf16); 1024-col superchunks
                energy = [enp.tile([P, S], BF16, tag=f"energy_{j}", name=f"energy_{j}_{b}") for j in range(2)]
                for sc in range(4):
                    for j in range(2):
                        ps = psk.tile([P, 1024], F32, tag="psk")
                        for h in range(2):
                            sl = slice(sc * 1024 + h * 512, sc * 1024 + (h + 1) * 512)
                            psl = slice(h * 512, (h + 1) * 512)
                            nc.tensor.matmul(ps[:, psl], w2_sb[0][j][:], encT_sb[0][:, sl],
                                             start=True, stop=False)
                            nc.tensor.matmul(ps[:, psl], w2_sb[1][j][:], encT_sb[1][:, sl],
                                             start=False, stop=True)
                        sl = slice(sc * 1024, (sc + 1) * 1024)
                        nc.scalar.activation(energy[j][:, sl], ps[:],
                                             mybir.ActivationFunctionType.Tanh,
                                             bias=qbt_sb[j][:, b:b + 1])

                # enc_nat deferred: only the context matmul needs it
                enc_nat_sb = encp.tile([P, NS * H], BF16, tag="enc_nat")
                for h in range(4):
                    sl = slice(h * 2048, (h + 1) * 2048)
                    nc.sync.dma_start(enc_nat_sb[:, sl], enc_nat[b][:, sl])
                return energy, enc_nat_sb

            def pass2(b, energy, enc_nat_sb):
                # scores: [s=128p, 32] via energy-stationary matmuls
                ps_sc = pss.tile([P, NS], F32, tag="ps_sc")
                for m in range(NS):
                    sl = slice(m * P, (m + 1) * P)
                    nc.tensor.matmul(ps_sc[:, m:m + 1], energy[0][:, sl], v_sb[0][:],
                                     start=True, stop=False)
                    nc.tensor.matmul(ps_sc[:, m:m + 1], energy[1][:, sl], v_sb[1][:],
                                     start=False, stop=True)

                # softmax (no max-subtract: |scores| <= 16)
                p_sb = smp.tile([P, NS], F32, tag="p_sb")
                rowsum = smp.tile([P, 1], F32, tag="rowsum")
                nc.scalar.activation(p_sb[:], ps_sc[:],
                                     mybir.ActivationFunctionType.Exp,
                                     accum_out=rowsum[:])
                ps_tot = psm.tile([1, 1], F32, tag="misc")
                nc.tensor.matmul(ps_tot[:], ones_col[:], rowsum[:])
                inv_sb = smp.tile([1, 1], F32, tag="inv_sb")
                nc.vector.reciprocal(inv_sb[:], ps_tot[:])
                ps_bc = psm.tile([P, 1], F32, tag="misc")
                nc.tensor.matmul(ps_bc[:], ones_row[:], inv_sb[:])
                inv_bc = smp.tile([P, 1], F32, tag="inv_bc")
                nc.vector.tensor_copy(inv_bc[:], ps_bc[:])

                attn_f = smp.tile([P, NS], F32, tag="attn_f")
                nc.vector.tensor_scalar_mul(attn_f[:], p_sb[:], inv_bc[:])
                attn_bf = smp.tile([P, NS], BF16, tag="attn_bf")
                nc.vector.tensor_copy(attn_bf[:], attn_f[:])

                # context: accumulate attn-weighted sum of enc rows
                ps_ctx = psm.tile([1, H], F32, tag="misc")
                for m in range(NS):
                    nc.tensor.matmul(ps_ctx[:], attn_bf[:, m:m + 1],
                                     enc_nat_sb[:, m * H:(m + 1) * H],
                                     start=(m == 0), stop=(m == NS - 1))
                ctx_sb = smp.tile([1, H], F32, tag="ctx_sb")
                nc.vector.tensor_copy(ctx_sb[:], ps_ctx[:])
                nc.sync.dma_start(ctx_out[b:b + 1, :], ctx_sb[:])

                # attn weights out: transpose to row-major then store
                ps_t = psm.tile([NS, P], F32, tag="misc")
                nc.tensor.transpose(ps_t[:], attn_f[:], ident_sb[:])
                attn_row = smp.tile([NS, P], F32, tag="attn_row")
                nc.vector.tensor_copy(attn_row[:], ps_t[:])
                nc.sync.dma_start(attn_out[b], attn_row[:])

            live = []
            for b in range(BL):
                live.append((b, *pass1(b)))
                if b >= 1:
                    pass2(*live.pop(0))
            pass2(*live.pop(0))

    nc.compile()
    return nc


def kernel(hidden, encoder_outputs, W1_w, W1_b, W2_w, W2_b, V_w, V_b):
    hidden = np.asarray(hidden, np.float32)
    enc = np.asarray(encoder_outputs, np.float32)

    # host-side prep (layout + tiny GEMM); V_b cancels in the softmax
    qb = (hidden @ np.asarray(W1_w, np.float32)
          + np.asarray(W1_b, np.float32) + np.asarray(W2_b, np.float32))  # (B, H)
    qbt = np.ascontiguousarray(qb.T)                                       # (H, B)
    enc_nat = np.ascontiguousarray(
        enc.reshape(B, NS, P, H).transpose(0, 2, 1, 3)                     # (B,P,NS,H)
    ).reshape(B, P, NS * H).astype(np.dtype("bfloat16"))
    encT = np.ascontiguousarray(enc.transpose(0, 2, 1)).reshape(
        B, 2, P, S).astype(np.dtype("bfloat16"))
    w2_bf = np.asarray(W2_w, np.float32).astype(np.dtype("bfloat16"))
    v_bf = np.asarray(V_w, np.float32).reshape(H, 1).astype(np.dtype("bfloat16"))
    ident = np.eye(P, dtype=np.float32)

    if "nc" not in _CACHE:
        _CACHE["nc"] = _build()
    nc = _CACHE["nc"]

    in_maps = []
    for c in range(NCORES):
        lo = c * BL
        in_maps.append({
            "enc_nat": enc_nat[lo:lo + BL],
            "encT": encT[lo:lo + BL],
            "w2": w2_bf,
            "vw": v_bf,
            "qbt": np.ascontiguousarray(qbt[:, lo:lo + BL]),
            "ident": ident,
        })

    global LAST_EXEC_NS, LAST_TRACE_DIR
    import tempfile
    kw = {}
    if TRACE:
        kw = dict(trace=True, tmpdir=tempfile.mkdtemp(prefix="bahdanau_ntff_"))
    res = run_bass_kernel_spmd(nc, in_maps, list(range(NCORES)), **kw)
    LAST_EXEC_NS = res.exec_time_ns
    LAST_TRACE_DIR = kw.get("tmpdir")

    attn = np.empty((B, S, 1), np.float32)
    ctxv = np.empty((B, H), np.float32)
    for c in range(NCORES):
        lo = c * BL
        attn[lo:lo + BL] = np.asarray(res.results[c]["attn_out"]).reshape(BL, S, 1)
        ctxv[lo:lo + BL] = np.asarray(res.results[c]["ctx_out"])
    return attn, ctxv


# revision 15
# speedup vs baseline: 1.0756x; 1.0756x over previous
"""Bahdanau attention on 8 trn2 NeuronCores, data-parallel over batch.

Per batch item (S=4096, H=256):
  k^T[h,s]  = sum_d W2[d,h] * encT[d,s]            (PE, bf16, fp32 accum)
  energyT   = tanh(k^T + (hidden@W1 + b1 + b2)[h]) (ACT, bias-folded)
  scores    = energyT^T @ V       -> [s=128p, 32]  (PE, energy as stationary)
  softmax   = exp + row-sum accum + ones-matmul partition sum (fp32)
  context   = sum_s attn[s] * enc[s,:]             (PE, attn cols as stationary)

Host precomputes q-bias (hidden@W1_w + W1_b + W2_b), pre-transposes
encoder_outputs, and casts the big operands to bf16. V_b cancels in softmax.
"""

import sys

import numpy as np

try:
    import concourse.bass as bass
except ImportError:
    sys.path.insert(0, "/opt/trn_rl_repo")
    import concourse.bass as bass

import concourse.tile as tile
from concourse import bacc, mybir
from concourse.bass_utils import run_bass_kernel_spmd

B, S, H = 32, 4096, 256
NCORES = 8
BL = B // NCORES          # batch items per core
P = 128                   # partitions
NS = S // P               # 32 s-blocks of 128
NC5 = S // 512            # 8 s-chunks of 512

F32 = mybir.dt.float32
BF16 = mybir.dt.bfloat16

_CACHE = {}
TRACE = False           # set by test harness to capture an NTFF profile
LAST_EXEC_NS = None
LAST_TRACE_DIR = None


def _build():
    nc = bacc.Bacc("TRN2", target_bir_lowering=False, debug=False,
                   num_devices=NCORES)
    enc_nat = nc.declare_dram_parameter("enc_nat", [BL, P, NS * H], BF16, isOutput=False)
    encT = nc.declare_dram_parameter("encT", [BL, 2, P, S], BF16, isOutput=False)
    w2p = nc.declare_dram_parameter("w2p", [P, 4 * P], BF16, isOutput=False)
    vp = nc.declare_dram_parameter("vp", [P, 2], BF16, isOutput=False)
    aux = nc.declare_dram_parameter("aux", [P, P + 2 * BL], F32, isOutput=False)
    attn_out = nc.declare_dram_parameter("attn_out", [BL, NS, P], F32, isOutput=True)
    ctx_out = nc.declare_dram_parameter("ctx_out", [BL, H], F32, isOutput=True)

    with tile.TileContext(nc) as tc:
        with (
            tc.tile_pool(name="singles", bufs=1) as singles,
            tc.tile_pool(name="enc", bufs=2) as encp,
            tc.tile_pool(name="energy", bufs=2) as enp,
            tc.tile_pool(name="sm", bufs=2) as smp,
            tc.tile_pool(name="psk", bufs=2, space="PSUM") as psk,
            tc.tile_pool(name="pss", bufs=2, space="PSUM") as pss,
            tc.tile_pool(name="psm", bufs=1, space="PSUM") as psm,
        ):
            # --- prologue: host-packed small operands, 3 DMAs ---
            w2p_sb = singles.tile([P, 4 * P], BF16, tag="w2p")
            nc.sync.dma_start(w2p_sb[:], w2p[:])
            vp_sb = singles.tile([P, 2], BF16, tag="vp")
            nc.sync.dma_start(vp_sb[:], vp[:])
            aux_sb = singles.tile([P, P + 2 * BL], F32, tag="aux")
            nc.sync.dma_start(aux_sb[:], aux[:])
            # views: w2 chunk (d_i, h_j) at cols (2i+j)*128; ident; qbt
            w2_sb = [[w2p_sb[:, (2 * i + j) * P:(2 * i + j + 1) * P] for j in range(2)]
                     for i in range(2)]
            v_sb = [vp_sb[:, j:j + 1] for j in range(2)]
            ident_sb = aux_sb[:, 0:P]
            qbt_sb = [aux_sb[:, P + j * BL:P + (j + 1) * BL] for j in range(2)]
            ones_col = singles.tile([P, 1], F32, tag="ones_col")
            nc.vector.memset(ones_col[:], 1.0)
            ones_row = singles.tile([1, P], F32, tag="ones_row")
            nc.vector.memset(ones_row[:], 1.0)

            def pass1(b):
                # encT first (k-projection consumes it), in 512K slices
                encT_sb = [encp.tile([P, S], BF16, tag=f"encT_{i}", bufs=3, name=f"encT_{i}_{b}") for i in range(2)]
                for i in range(2):
                    for h in range(2):
                        sl = slice(h * 2048, (h + 1) * 2048)
                        nc.sync.dma_start(encT_sb[i][:, sl], encT[b, i][:, sl])

                # k^T then tanh(..+q) -> energyT (bf16); 1024-col superchunks
                energy = [enp.tile([P, S], BF16, tag=f"energy_{j}", name=f"energy_{j}_{b}") for j in range(2)]
                for sc in range(4):
                    for j in range(2):
                        ps = psk.tile([P, 1024], F32, tag="psk")
                        for h in range(2):
                            sl = slice(sc * 1024 + h * 512, sc * 1024 + (h + 1) * 512)
                            psl = slice(h * 512, (h + 1) * 512)
                            nc.tensor.matmul(ps[:, psl], w2_sb[0][j], encT_sb[0][:, sl],
                                             start=True, stop=False)
                            nc.tensor.matmul(ps[:, psl], w2_sb[1][j], encT_sb[1][:, sl],
                                             start=False, stop=True)
                        sl = slice(sc * 1024, (sc + 1) * 1024)
                        nc.scalar.activation(energy[j][:, sl], ps[:],
                                             mybir.ActivationFunctionType.Tanh,
                                             bias=qbt_sb[j][:, b:b + 1])

                # enc_nat deferred: only the context matmul needs it
                enc_nat_sb = encp.tile([P, NS * H], BF16, tag="enc_nat", bufs=4, name=f"enc_nat_{b}")
                for h in range(4):
                    sl = slice(h * 2048, (h + 1) * 2048)
                    nc.sync.dma_start(enc_nat_sb[:, sl], enc_nat[b][:, sl])
                return energy, enc_nat_sb

            def scores(b, energy):
                ps_sc = pss.tile([P, NS], F32, tag="ps_sc", name=f"ps_sc_{b}")
                for m in range(NS):
                    sl = slice(m * P, (m + 1) * P)
                    nc.tensor.matmul(ps_sc[:, m:m + 1], energy[0][:, sl], v_sb[0],
                                     start=True, stop=False)
                    nc.tensor.matmul(ps_sc[:, m:m + 1], energy[1][:, sl], v_sb[1],
                                     start=False, stop=True)
                return ps_sc

            def finish(b, ps_sc, enc_nat_sb):
                # softmax (no max-subtract: |scores| <= 16)
                p_sb = smp.tile([P, NS], F32, tag="p_sb", name=f"p_sb_{b}")
                rowsum = smp.tile([P, 1], F32, tag="rowsum", name=f"rowsum_{b}")
                nc.scalar.activation(p_sb[:], ps_sc[:],
                                     mybir.ActivationFunctionType.Exp,
                                     accum_out=rowsum[:])
                ps_tot = psm.tile([1, 1], F32, tag="misc", name=f"ps_tot_{b}")
                nc.tensor.matmul(ps_tot[:], ones_col[:], rowsum[:])
                inv_sb = smp.tile([1, 1], F32, tag="inv_sb", name=f"inv_sb_{b}")
                nc.vector.reciprocal(inv_sb[:], ps_tot[:])
                ps_bc = psm.tile([P, 1], F32, tag="misc", name=f"ps_bc_{b}")
                nc.tensor.matmul(ps_bc[:], ones_row[:], inv_sb[:])
                inv_bc = smp.tile([P, 1], F32, tag="inv_bc", name=f"inv_bc_{b}")
                nc.vector.tensor_copy(inv_bc[:], ps_bc[:])

                attn_f = smp.tile([P, NS], F32, tag="attn_f", name=f"attn_f_{b}")
                nc.vector.tensor_scalar_mul(attn_f[:], p_sb[:], inv_bc[:])
                attn_bf = smp.tile([P, NS], BF16, tag="attn_bf", name=f"attn_bf_{b}")
                nc.vector.tensor_copy(attn_bf[:], attn_f[:])

                # context: accumulate attn-weighted sum of enc rows
                ps_ctx = psm.tile([1, H], F32, tag="misc", name=f"ps_ctx_{b}")
                for m in range(NS):
                    nc.tensor.matmul(ps_ctx[:], attn_bf[:, m:m + 1],
                                     enc_nat_sb[:, m * H:(m + 1) * H],
                                     start=(m == 0), stop=(m == NS - 1))
                ctx_sb = smp.tile([1, H], F32, tag="ctx_sb", name=f"ctx_sb_{b}")
                nc.vector.tensor_copy(ctx_sb[:], ps_ctx[:])
                nc.sync.dma_start(ctx_out[b:b + 1, :], ctx_sb[:])

                # attn weights out: transpose to row-major then store
                ps_t = psm.tile([NS, P], F32, tag="misc", name=f"ps_t_{b}")
                nc.tensor.transpose(ps_t[:], attn_f[:], ident_sb)
                attn_row = smp.tile([NS, P], F32, tag="attn_row", name=f"attn_row_{b}")
                nc.vector.tensor_copy(attn_row[:], ps_t[:])
                nc.sync.dma_start(attn_out[b], attn_row[:])

            # software pipeline: PE order interleaves item b's scores/context
            # behind item b+1's k-projection so ACT always has tanh work
            state = {}
            for b in range(BL):
                state[b] = pass1(b)             # (energy, enc_nat)
                if b >= 1:
                    state[b - 1] = (*state[b - 1], scores(b - 1, state[b - 1][0]))
                if b >= 2:
                    e, nat, sc = state.pop(b - 2)
                    finish(b - 2, sc, nat)
            state[BL - 1] = (*state[BL - 1], scores(BL - 1, state[BL - 1][0]))
            for b in (BL - 2, BL - 1):
                e, nat, sc = state.pop(b)
                finish(b, sc, nat)

    nc.compile()
    return nc


def kernel(hidden, encoder_outputs, W1_w, W1_b, W2_w, W2_b, V_w, V_b):
    hidden = np.asarray(hidden, np.float32)
    enc = np.asarray(encoder_outputs, np.float32)

    # host-side prep (layout + tiny GEMM); V_b cancels in the softmax
    qb = (hidden @ np.asarray(W1_w, np.float32)
          + np.asarray(W1_b, np.float32) + np.asarray(W2_b, np.float32))  # (B, H)
    qbt = np.ascontiguousarray(qb.T)                                       # (H, B)
    enc_nat = np.ascontiguousarray(
        enc.reshape(B, NS, P, H).transpose(0, 2, 1, 3)                     # (B,P,NS,H)
    ).reshape(B, P, NS * H).astype(np.dtype("bfloat16"))
    encT = np.ascontiguousarray(enc.transpose(0, 2, 1)).reshape(
        B, 2, P, S).astype(np.dtype("bfloat16"))
    # packed prologue operands: w2p[p, (2i+j)*128+c] = W2[128i+p, 128j+c]
    w2p = np.ascontiguousarray(
        np.asarray(W2_w, np.float32).reshape(2, P, 2, P).transpose(1, 0, 2, 3)
    ).reshape(P, 4 * P).astype(np.dtype("bfloat16"))
    vp = np.ascontiguousarray(
        np.asarray(V_w, np.float32).reshape(2, P).T).astype(np.dtype("bfloat16"))
    ident = np.eye(P, dtype=np.float32)

    if "nc" not in _CACHE:
        _CACHE["nc"] = _build()
    nc = _CACHE["nc"]

    in_maps = []
    for c in range(NCORES):
        lo = c * BL
        # aux = [ident | qbt chunk0 | qbt chunk1], per-core
        qslab = qbt[:, lo:lo + BL].reshape(2, P, BL)
        aux = np.concatenate([ident, qslab[0], qslab[1]], axis=1).astype(np.float32)
        in_maps.append({
            "enc_nat": enc_nat[lo:lo + BL],
            "encT": encT[lo:lo + BL],
            "w2p": w2p,
            "vp": vp,
            "aux": np.ascontiguousarray(aux),
        })

    global LAST_EXEC_NS, LAST_TRACE_DIR
    import tempfile
    kw = {}
    if TRACE:
        kw = dict(trace=True, tmpdir=tempfile.mkdtemp(prefix="bahdanau_ntff_"))
    res = run_bass_kernel_spmd(nc, in_maps, list(range(NCORES)), **kw)
    LAST_EXEC_NS = res.exec_time_ns
    LAST_TRACE_DIR = kw.get("tmpdir")

    attn = np.empty((B, S, 1), np.float32)
    ctxv = np.empty((B, H), np.float32)
    for c in range(NCORES):
        lo = c * BL
        attn[lo:lo + BL] = np.asarray(res.results[c]["attn_out"]).reshape(BL, S, 1)
        ctxv[lo:lo + BL] = np.asarray(res.results[c]["ctx_out"])
    return attn, ctxv


# revision 17
# speedup vs baseline: 1.1476x; 1.0669x over previous
"""Bahdanau attention on 8 trn2 NeuronCores, data-parallel over batch.

Per batch item (S=4096, H=256):
  k^T[h,s]  = sum_d W2[d,h] * encT[d,s]            (PE, bf16, fp32 accum)
  energyT   = tanh(k^T + (hidden@W1 + b1 + b2)[h]) (ACT, bias-folded)
  scores    = energyT^T @ V       -> [s=128p, 32]  (PE, energy as stationary)
  softmax   = exp + row-sum accum + ones-matmul partition sum (fp32)
  context   = sum_s attn[s] * enc[s,:]             (PE, attn cols as stationary)

Host precomputes q-bias (hidden@W1_w + W1_b + W2_b), pre-transposes
encoder_outputs, and casts the big operands to bf16. V_b cancels in softmax.
"""

import sys

import numpy as np

try:
    import concourse.bass as bass
except ImportError:
    sys.path.insert(0, "/opt/trn_rl_repo")
    import concourse.bass as bass

import concourse.tile as tile
from concourse import bacc, mybir
from concourse.bass_utils import run_bass_kernel_spmd

B, S, H = 32, 4096, 256
NCORES = 8
BL = B // NCORES          # batch items per core
P = 128                   # partitions
NS = S // P               # 32 s-blocks of 128
NC5 = S // 512            # 8 s-chunks of 512

F32 = mybir.dt.float32
BF16 = mybir.dt.bfloat16
FP16 = mybir.dt.float16

_CACHE = {}
TRACE = False           # set by test harness to capture an NTFF profile
LAST_EXEC_NS = None
LAST_TRACE_DIR = None


def _build():
    nc = bacc.Bacc("TRN2", target_bir_lowering=False, debug=False,
                   num_devices=NCORES)
    enc_nat = nc.declare_dram_parameter("enc_nat", [BL, P, NS * H], BF16, isOutput=False)
    encT = nc.declare_dram_parameter("encT", [BL, 2, P, S], BF16, isOutput=False)
    w2p = nc.declare_dram_parameter("w2p", [P, 4 * P], BF16, isOutput=False)
    vp = nc.declare_dram_parameter("vp", [P, 2], BF16, isOutput=False)
    aux = nc.declare_dram_parameter("aux", [P, P + 2 * BL + 2], F32, isOutput=False)
    attn_out = nc.declare_dram_parameter("attn_out", [BL, NS, P], F32, isOutput=True)
    ctx_out = nc.declare_dram_parameter("ctx_out", [BL, H], F32, isOutput=True)

    with tile.TileContext(nc) as tc:
        with (
            tc.tile_pool(name="singles", bufs=1) as singles,
            tc.tile_pool(name="enc", bufs=2) as encp,
            tc.tile_pool(name="energy", bufs=2) as enp,
            tc.tile_pool(name="evs", bufs=2) as evs,
            tc.tile_pool(name="sm", bufs=2) as smp,
            tc.tile_pool(name="psk", bufs=2, space="PSUM") as psk,
            tc.tile_pool(name="pss", bufs=2, space="PSUM") as pss,
            tc.tile_pool(name="psm", bufs=1, space="PSUM") as psm,
        ):
            # --- prologue: host-packed small operands, 3 DMAs on the ACT
            # HWDGE ring so they don't delay the enc stream on the SP ring
            w2p_sb = singles.tile([P, 4 * P], BF16, tag="w2p")
            nc.scalar.dma_start(w2p_sb[:], w2p[:])
            vp_sb = singles.tile([P, 2], BF16, tag="vp")
            nc.scalar.dma_start(vp_sb[:], vp[:])
            aux_sb = singles.tile([P, P + 2 * BL + 2], F32, tag="aux")
            nc.scalar.dma_start(aux_sb[:], aux[:])
            # views: w2 chunk (d_i, h_j) at cols (2i+j)*128; ident; qbt; V f32
            w2_sb = [[w2p_sb[:, (2 * i + j) * P:(2 * i + j + 1) * P] for j in range(2)]
                     for i in range(2)]
            ident_sb = aux_sb[:, 0:P]
            qbt_sb = [aux_sb[:, P + j * BL:P + (j + 1) * BL] for j in range(2)]
            vf_sb = [aux_sb[:, P + 2 * BL + j:P + 2 * BL + j + 1] for j in range(2)]
            ones_col = singles.tile([P, 1], F32, tag="ones_col")
            nc.vector.memset(ones_col[:], 1.0)
            ones_row = singles.tile([1, P], F32, tag="ones_row")
            nc.vector.memset(ones_row[:], 1.0)
            ones_sc = singles.tile([P, 1], FP16, tag="ones_sc")
            nc.vector.memset(ones_sc[:], 1.0)
            ones97 = singles.tile([3 * 32 + 1, 1], F32, tag="ones97")
            nc.vector.memset(ones97[:], 1.0)

            def pass1(b):
                # encT: 8 x 512K slices, d-chunks interleaved so the first
                # k-projection superchunk can start after two slices
                encT_sb = [encp.tile([P, S], BF16, tag=f"encT_{i}", bufs=3, name=f"encT_{i}_{b}") for i in range(2)]
                for h in range(4):
                    sl = slice(h * 1024, (h + 1) * 1024)
                    for i in range(2):
                        nc.sync.dma_start(encT_sb[i][:, sl], encT[b, i][:, sl])

                # k^T -> tanh(..+q) -> energy (fp16) -> EV = V0*E0 + V1*E1 (fp16)
                energy = [enp.tile([P, S], FP16, tag=f"energy_{j}", name=f"energy_{j}_{b}") for j in range(2)]
                ev = enp.tile([P, S], FP16, tag="ev", name=f"ev_{b}")
                for sc in range(4):
                    for j in range(2):
                        ps = psk.tile([P, 1024], F32, tag="psk")
                        for h in range(2):
                            sl = slice(sc * 1024 + h * 512, sc * 1024 + (h + 1) * 512)
                            psl = slice(h * 512, (h + 1) * 512)
                            nc.tensor.matmul(ps[:, psl], w2_sb[0][j], encT_sb[0][:, sl],
                                             start=True, stop=False)
                            nc.tensor.matmul(ps[:, psl], w2_sb[1][j], encT_sb[1][:, sl],
                                             start=False, stop=True)
                        sl = slice(sc * 1024, (sc + 1) * 1024)
                        nc.scalar.activation(energy[j][:, sl], ps[:],
                                             mybir.ActivationFunctionType.Tanh,
                                             bias=qbt_sb[j][:, b:b + 1])
                    sl = slice(sc * 1024, (sc + 1) * 1024)
                    tmp = evs.tile([P, 1024], FP16, tag="tmp", name=f"tmp_{b}_{sc}")
                    nc.vector.tensor_scalar_mul(tmp[:], energy[0][:, sl], vf_sb[0])
                    nc.vector.tensor_scalar_mul(ev[:, sl], energy[1][:, sl], vf_sb[1])
                    nc.vector.tensor_add(ev[:, sl], ev[:, sl], tmp[:])

                # enc_nat deferred: only the context matmul needs it
                enc_nat_sb = encp.tile([P, NS * H], BF16, tag="enc_nat", bufs=3, name=f"enc_nat_{b}")
                for h in range(4):
                    sl = slice(h * 2048, (h + 1) * 2048)
                    nc.sync.dma_start(enc_nat_sb[:, sl], enc_nat[b][:, sl])
                return ev, enc_nat_sb

            def scores(b, ev):
                ps_sc = pss.tile([P, NS], F32, tag="ps_sc", name=f"ps_sc_{b}")
                for m in range(NS):
                    nc.tensor.matmul(ps_sc[:, m:m + 1], ev[:, m * P:(m + 1) * P],
                                     ones_sc[:], start=True, stop=True)
                return ps_sc

            def finish(b, ps_sc, enc_nat_sb):
                # softmax (no max-subtract: |scores| <= 16)
                p_sb = smp.tile([P, NS], F32, tag="p_sb", name=f"p_sb_{b}")
                rowsum = smp.tile([P, 1], F32, tag="rowsum", name=f"rowsum_{b}")
                nc.scalar.activation(p_sb[:], ps_sc[:],
                                     mybir.ActivationFunctionType.Exp,
                                     accum_out=rowsum[:])
                ps_tot = psm.tile([1, 1], F32, tag="misc", name=f"ps_tot_{b}")
                nc.tensor.matmul(ps_tot[:], ones_col[:], rowsum[:])
                inv_sb = smp.tile([1, 1], F32, tag="inv_sb", name=f"inv_sb_{b}")
                nc.vector.reciprocal(inv_sb[:], ps_tot[:])
                ps_bc = psm.tile([P, 1], F32, tag="misc", name=f"ps_bc_{b}")
                nc.tensor.matmul(ps_bc[:], ones_row[:], inv_sb[:])
                inv_bc = smp.tile([P, 1], F32, tag="inv_bc", name=f"inv_bc_{b}")
                nc.vector.tensor_copy(inv_bc[:], ps_bc[:])

                attn_f = smp.tile([P, NS], F32, tag="attn_f", name=f"attn_f_{b}")
                nc.vector.tensor_scalar_mul(attn_f[:], p_sb[:], inv_bc[:])
                attn_bf = smp.tile([P, NS], BF16, tag="attn_bf", name=f"attn_bf_{b}")
                nc.vector.tensor_copy(attn_bf[:], attn_f[:])

                # context: 4 concurrent col-group accumulators, then a
                # 97-row ones-matmul folds the partials (memset rows read 0)
                ps_c = psm.tile([97, H], F32, tag="misc", name=f"ps_c_{b}")
                nc.vector.memset(ps_c[:], 0.0)
                for k in range(8):
                    for g in range(4):
                        m = 8 * g + k
                        nc.tensor.matmul(ps_c[32 * g:32 * g + 1, :],
                                         attn_bf[:, m:m + 1],
                                         enc_nat_sb[:, m * H:(m + 1) * H],
                                         start=(k == 0), stop=(k == 7),
                                         tile_position=(0, 32 * g))
                ctx97 = smp.tile([97, H], F32, tag="ctx97", name=f"ctx97_{b}")
                nc.vector.tensor_copy(ctx97[:], ps_c[:])
                ps_ctx = psm.tile([1, H], F32, tag="misc", name=f"ps_ctx_{b}")
                nc.tensor.matmul(ps_ctx[:], ones97[:], ctx97[:])
                ctx_sb = smp.tile([1, H], F32, tag="ctx_sb", name=f"ctx_sb_{b}")
                nc.vector.tensor_copy(ctx_sb[:], ps_ctx[:])
                nc.sync.dma_start(ctx_out[b:b + 1, :], ctx_sb[:])

                # attn weights out: transpose to row-major then store
                ps_t = psm.tile([NS, P], F32, tag="misc", name=f"ps_t_{b}")
                nc.tensor.transpose(ps_t[:], attn_f[:], ident_sb)
                attn_row = smp.tile([NS, P], F32, tag="attn_row", name=f"attn_row_{b}")
                nc.vector.tensor_copy(attn_row[:], ps_t[:])
                nc.sync.dma_start(attn_out[b], attn_row[:])

            # software pipeline: PE order interleaves item b's scores/context
            # behind item b+1's k-projection so ACT always has tanh work
            state = {}
            for b in range(BL):
                state[b] = pass1(b)             # (energy, enc_nat)
                if b >= 1:
                    state[b - 1] = (*state[b - 1], scores(b - 1, state[b - 1][0]))
                if b >= 2:
                    e, nat, sc = state.pop(b - 2)
                    finish(b - 2, sc, nat)
            state[BL - 1] = (*state[BL - 1], scores(BL - 1, state[BL - 1][0]))
            for b in (BL - 2, BL - 1):
                e, nat, sc = state.pop(b)
                finish(b, sc, nat)

    nc.compile()
    return nc


def kernel(hidden, encoder_outputs, W1_w, W1_b, W2_w, W2_b, V_w, V_b):
    hidden = np.asarray(hidden, np.float32)
    enc = np.asarray(encoder_outputs, np.float32)

    # host-side prep (layout + tiny GEMM); V_b cancels in the softmax
    qb = (hidden @ np.asarray(W1_w, np.float32)
          + np.asarray(W1_b, np.float32) + np.asarray(W2_b, np.float32))  # (B, H)
    qbt = np.ascontiguousarray(qb.T)                                       # (H, B)
    enc_nat = np.ascontiguousarray(
        enc.reshape(B, NS, P, H).transpose(0, 2, 1, 3)                     # (B,P,NS,H)
    ).reshape(B, P, NS * H).astype(np.dtype("bfloat16"))
    encT = np.ascontiguousarray(enc.transpose(0, 2, 1)).reshape(
        B, 2, P, S).astype(np.dtype("bfloat16"))
    # packed prologue operands: w2p[p, (2i+j)*128+c] = W2[128i+p, 128j+c]
    w2p = np.ascontiguousarray(
        np.asarray(W2_w, np.float32).reshape(2, P, 2, P).transpose(1, 0, 2, 3)
    ).reshape(P, 4 * P).astype(np.dtype("bfloat16"))
    vp = np.ascontiguousarray(
        np.asarray(V_w, np.float32).reshape(2, P).T).astype(np.dtype("bfloat16"))
    ident = np.eye(P, dtype=np.float32)

    if "nc" not in _CACHE:
        _CACHE["nc"] = _build()
    nc = _CACHE["nc"]

    in_maps = []
    for c in range(NCORES):
        lo = c * BL
        # aux = [ident | qbt chunk0 | qbt chunk1], per-core
        qslab = qbt[:, lo:lo + BL].reshape(2, P, BL)
        vf = np.asarray(V_w, np.float32).reshape(2, P).T
        aux = np.concatenate([ident, qslab[0], qslab[1], vf], axis=1).astype(np.float32)
        in_maps.append({
            "enc_nat": enc_nat[lo:lo + BL],
            "encT": encT[lo:lo + BL],
            "w2p": w2p,
            "vp": vp,
            "aux": np.ascontiguousarray(aux),
        })

    global LAST_EXEC_NS, LAST_TRACE_DIR
    import tempfile
    kw = {}
    if TRACE:
        kw = dict(trace=True, tmpdir=tempfile.mkdtemp(prefix="bahdanau_ntff_"))
    res = run_bass_kernel_spmd(nc, in_maps, list(range(NCORES)), **kw)
    LAST_EXEC_NS = res.exec_time_ns
    LAST_TRACE_DIR = kw.get("tmpdir")

    attn = np.empty((B, S, 1), np.float32)
    ctxv = np.empty((B, H), np.float32)
    for c in range(NCORES):
        lo = c * BL
        attn[lo:lo + BL] = np.asarray(res.results[c]["attn_out"]).reshape(BL, S, 1)
        ctxv[lo:lo + BL] = np.asarray(res.results[c]["ctx_out"])
    return attn, ctxv


# revision 18
# speedup vs baseline: 1.1796x; 1.0279x over previous
"""Bahdanau attention on 8 trn2 NeuronCores, data-parallel over batch.

Per batch item (S=4096, H=256):
  k^T[h,s]  = sum_d W2[d,h] * encT[d,s]            (PE, bf16, fp32 accum)
  energyT   = tanh(k^T + (hidden@W1 + b1 + b2)[h]) (ACT, bias-folded)
  scores    = energyT^T @ V       -> [s=128p, 32]  (PE, energy as stationary)
  softmax   = exp + row-sum accum + ones-matmul partition sum (fp32)
  context   = sum_s attn[s] * enc[s,:]             (PE, attn cols as stationary)

Host precomputes q-bias (hidden@W1_w + W1_b + W2_b), pre-transposes
encoder_outputs, and casts the big operands to bf16. V_b cancels in softmax.
"""

import sys

import numpy as np

try:
    import concourse.bass as bass
except ImportError:
    sys.path.insert(0, "/opt/trn_rl_repo")
    import concourse.bass as bass

import concourse.tile as tile
from concourse import bacc, mybir
from concourse.bass_utils import run_bass_kernel_spmd

B, S, H = 32, 4096, 256
NCORES = 8
BL = B // NCORES          # batch items per core
P = 128                   # partitions
NS = S // P               # 32 s-blocks of 128
NC5 = S // 512            # 8 s-chunks of 512

F32 = mybir.dt.float32
BF16 = mybir.dt.bfloat16
FP16 = mybir.dt.float16

_CACHE = {}
TRACE = False           # set by test harness to capture an NTFF profile
LAST_EXEC_NS = None
LAST_TRACE_DIR = None


def _build():
    nc = bacc.Bacc("TRN2", target_bir_lowering=False, debug=False,
                   num_devices=NCORES)
    enc_nat = nc.declare_dram_parameter("enc_nat", [BL, P, NS * H], BF16, isOutput=False)
    encT = nc.declare_dram_parameter("encT", [BL, 2, P, S], BF16, isOutput=False)
    w2p = nc.declare_dram_parameter("w2p", [P, 4 * P], BF16, isOutput=False)
    vp = nc.declare_dram_parameter("vp", [P, 2], BF16, isOutput=False)
    aux = nc.declare_dram_parameter("aux", [P, P + 2 * BL + 2], F32, isOutput=False)
    attn_out = nc.declare_dram_parameter("attn_out", [BL, NS, P], F32, isOutput=True)
    ctx_out = nc.declare_dram_parameter("ctx_out", [BL, H], F32, isOutput=True)

    with tile.TileContext(nc) as tc:
        with (
            tc.tile_pool(name="singles", bufs=1) as singles,
            tc.tile_pool(name="enc", bufs=2) as encp,
            tc.tile_pool(name="energy", bufs=2) as enp,
            tc.tile_pool(name="evs", bufs=2) as evs,
            tc.tile_pool(name="sm", bufs=2) as smp,
            tc.tile_pool(name="psk", bufs=2, space="PSUM") as psk,
            tc.tile_pool(name="pss", bufs=2, space="PSUM") as pss,
            tc.tile_pool(name="psc", bufs=1, space="PSUM") as pscp,
            tc.tile_pool(name="psm", bufs=1, space="PSUM") as psm,
        ):
            # --- prologue: host-packed small operands, 3 DMAs on the ACT
            # HWDGE ring so they don't delay the enc stream on the SP ring
            w2p_sb = singles.tile([P, 4 * P], BF16, tag="w2p")
            nc.scalar.dma_start(w2p_sb[:], w2p[:])
            vp_sb = singles.tile([P, 2], BF16, tag="vp")
            nc.scalar.dma_start(vp_sb[:], vp[:])
            aux_sb = singles.tile([P, P + 2 * BL + 2], F32, tag="aux")
            nc.scalar.dma_start(aux_sb[:], aux[:])
            w2_sb = [[w2p_sb[:, (2 * i + j) * P:(2 * i + j + 1) * P] for j in range(2)]
                     for i in range(2)]
            ident_sb = aux_sb[:, 0:P]
            qbt_sb = [aux_sb[:, P + j * BL:P + (j + 1) * BL] for j in range(2)]
            vf_sb = [aux_sb[:, P + 2 * BL + j:P + 2 * BL + j + 1] for j in range(2)]
            ones_col = singles.tile([P, 1], F32, tag="ones_col")
            nc.vector.memset(ones_col[:], 1.0)
            ones_row = singles.tile([1, P], F32, tag="ones_row")
            nc.vector.memset(ones_row[:], 1.0)
            ones_sc = singles.tile([P, 1], FP16, tag="ones_sc")
            nc.vector.memset(ones_sc[:], 1.0)
            ones97 = singles.tile([3 * 32 + 1, 1], F32, tag="ones97")
            nc.vector.memset(ones97[:], 1.0)

            st = {}   # per-item live state

            def softmax(b):
                s = st[b]
                p_sb = smp.tile([P, NS], F32, tag="p_sb", name=f"p_sb_{b}")
                rowsum = smp.tile([P, 1], F32, tag="rowsum", name=f"rowsum_{b}")
                nc.scalar.activation(p_sb[:], s["ps_sc"][:],
                                     mybir.ActivationFunctionType.Exp,
                                     accum_out=rowsum[:])
                ps_tot = psm.tile([1, 1], F32, tag="misc", name=f"ps_tot_{b}")
                nc.tensor.matmul(ps_tot[:], ones_col[:], rowsum[:])
                inv_sb = smp.tile([1, 1], F32, tag="inv_sb", name=f"inv_sb_{b}")
                nc.vector.reciprocal(inv_sb[:], ps_tot[:])
                ps_bc = psm.tile([P, 1], F32, tag="misc", name=f"ps_bc_{b}")
                nc.tensor.matmul(ps_bc[:], ones_row[:], inv_sb[:])
                inv_bc = smp.tile([P, 1], F32, tag="inv_bc", name=f"inv_bc_{b}")
                nc.vector.tensor_copy(inv_bc[:], ps_bc[:])
                attn_f = smp.tile([P, NS], F32, tag="attn_f", name=f"attn_f_{b}")
                nc.vector.tensor_scalar_mul(attn_f[:], p_sb[:], inv_bc[:])
                attn_bf = smp.tile([P, NS], BF16, tag="attn_bf", name=f"attn_bf_{b}")
                nc.vector.tensor_copy(attn_bf[:], attn_f[:])
                s["attn_f"], s["attn_bf"] = attn_f, attn_bf
                s["ps_c"] = pscp.tile([97, H], F32, tag="ps_c", name=f"ps_c_{b}")
                nc.vector.memset(s["ps_c"][:], 0.0)

            def emit_ctx(b, k):
                s = st[b]
                for g in range(4):
                    m = 8 * g + k
                    nc.tensor.matmul(s["ps_c"][32 * g:32 * g + 1, :],
                                     s["attn_bf"][:, m:m + 1],
                                     s["enc_nat"][:, m * H:(m + 1) * H],
                                     start=(k == 0), stop=(k == 7),
                                     tile_position=(0, 32 * g))

            def finish(b):
                s = st.pop(b)
                ctx97 = smp.tile([97, H], F32, tag="ctx97", name=f"ctx97_{b}")
                nc.vector.tensor_copy(ctx97[:], s["ps_c"][:])
                ps_ctx = psm.tile([1, H], F32, tag="misc", name=f"ps_ctx_{b}")
                nc.tensor.matmul(ps_ctx[:], ones97[:], ctx97[:])
                ctx_sb = smp.tile([1, H], F32, tag="ctx_sb", name=f"ctx_sb_{b}")
                nc.vector.tensor_copy(ctx_sb[:], ps_ctx[:])
                nc.sync.dma_start(ctx_out[b:b + 1, :], ctx_sb[:])
                ps_t = psm.tile([NS, P], F32, tag="misc", name=f"ps_t_{b}")
                nc.tensor.transpose(ps_t[:], s["attn_f"][:], ident_sb)
                attn_row = smp.tile([NS, P], F32, tag="attn_row", name=f"attn_row_{b}")
                nc.vector.tensor_copy(attn_row[:], ps_t[:])
                nc.sync.dma_start(attn_out[b], attn_row[:])

            # steady-state iteration: pass1 of item b interleaved at
            # superchunk grain with scores of b-1 and context of b-2,
            # so the in-order PE queue never stalls on ACT/PSUM deps.
            for b in range(BL + 2):
                p1 = b < BL
                if p1:
                    encT_sb = [encp.tile([P, S], BF16, tag=f"encT_{i}", bufs=3,
                                         name=f"encT_{i}_{b}") for i in range(2)]
                    for h in range(4):
                        sl = slice(h * 1024, (h + 1) * 1024)
                        for i in range(2):
                            nc.sync.dma_start(encT_sb[i][:, sl], encT[b, i][:, sl])
                    energy = [enp.tile([P, S], FP16, tag=f"energy_{j}",
                                       name=f"energy_{j}_{b}") for j in range(2)]
                    ev = enp.tile([P, S], FP16, tag="ev", name=f"ev_{b}")
                    st[b] = {"ev": ev}
                if b >= 2:
                    softmax(b - 2)
                if b >= 1 and b - 1 < BL:
                    st[b - 1]["ps_sc"] = pss.tile([P, NS], F32, tag="ps_sc",
                                                  name=f"ps_sc_{b-1}")
                for sc in range(4):
                    if p1:
                        for j in range(2):
                            ps = psk.tile([P, 1024], F32, tag="psk")
                            for h in range(2):
                                sl = slice(sc * 1024 + h * 512, sc * 1024 + (h + 1) * 512)
                                psl = slice(h * 512, (h + 1) * 512)
                                nc.tensor.matmul(ps[:, psl], w2_sb[0][j], encT_sb[0][:, sl],
                                                 start=True, stop=False)
                                nc.tensor.matmul(ps[:, psl], w2_sb[1][j], encT_sb[1][:, sl],
                                                 start=False, stop=True)
                            sl = slice(sc * 1024, (sc + 1) * 1024)
                            nc.scalar.activation(energy[j][:, sl], ps[:],
                                                 mybir.ActivationFunctionType.Tanh,
                                                 bias=qbt_sb[j][:, b:b + 1])
                        sl = slice(sc * 1024, (sc + 1) * 1024)
                        tmp = evs.tile([P, 1024], FP16, tag="tmp", name=f"tmp_{b}_{sc}")
                        nc.vector.tensor_scalar_mul(tmp[:], energy[0][:, sl], vf_sb[0])
                        nc.vector.tensor_scalar_mul(ev[:, sl], energy[1][:, sl], vf_sb[1])
                        nc.vector.tensor_add(ev[:, sl], ev[:, sl], tmp[:])
                    if b >= 1 and b - 1 < BL:
                        sp = st[b - 1]
                        for m in range(8 * sc, 8 * sc + 8):
                            nc.tensor.matmul(sp["ps_sc"][:, m:m + 1],
                                             sp["ev"][:, m * P:(m + 1) * P],
                                             ones_sc[:], start=True, stop=True)
                    if b >= 2:
                        emit_ctx(b - 2, 2 * sc)
                        emit_ctx(b - 2, 2 * sc + 1)
                if p1:
                    st[b]["enc_nat"] = encp.tile([P, NS * H], BF16, tag="enc_nat",
                                                 bufs=3, name=f"enc_nat_{b}")
                    for h in range(4):
                        sl = slice(h * 2048, (h + 1) * 2048)
                        nc.sync.dma_start(st[b]["enc_nat"][:, sl], enc_nat[b][:, sl])
                if b >= 2:
                    finish(b - 2)

    nc.compile()
    return nc


def kernel(hidden, encoder_outputs, W1_w, W1_b, W2_w, W2_b, V_w, V_b):
    hidden = np.asarray(hidden, np.float32)
    enc = np.asarray(encoder_outputs, np.float32)

    # host-side prep (layout + tiny GEMM); V_b cancels in the softmax
    qb = (hidden @ np.asarray(W1_w, np.float32)
          + np.asarray(W1_b, np.float32) + np.asarray(W2_b, np.float32))  # (B, H)
    qbt = np.ascontiguousarray(qb.T)                                       # (H, B)
    enc_nat = np.ascontiguousarray(
        enc.reshape(B, NS, P, H).transpose(0, 2, 1, 3)                     # (B,P,NS,H)
    ).reshape(B, P, NS * H).astype(np.dtype("bfloat16"))
    encT = np.ascontiguousarray(enc.transpose(0, 2, 1)).reshape(
        B, 2, P, S).astype(np.dtype("bfloat16"))
    # packed prologue operands: w2p[p, (2i+j)*128+c] = W2[128i+p, 128j+c]
    w2p = np.ascontiguousarray(
        np.asarray(W2_w, np.float32).reshape(2, P, 2, P).transpose(1, 0, 2, 3)
    ).reshape(P, 4 * P).astype(np.dtype("bfloat16"))
    vp = np.ascontiguousarray(
        np.asarray(V_w, np.float32).reshape(2, P).T).astype(np.dtype("bfloat16"))
    ident = np.eye(P, dtype=np.float32)

    if "nc" not in _CACHE:
        _CACHE["nc"] = _build()
    nc = _CACHE["nc"]

    in_maps = []
    for c in range(NCORES):
        lo = c * BL
        # aux = [ident | qbt chunk0 | qbt chunk1], per-core
        qslab = qbt[:, lo:lo + BL].reshape(2, P, BL)
        vf = np.asarray(V_w, np.float32).reshape(2, P).T
        aux = np.concatenate([ident, qslab[0], qslab[1], vf], axis=1).astype(np.float32)
        in_maps.append({
            "enc_nat": enc_nat[lo:lo + BL],
            "encT": encT[lo:lo + BL],
            "w2p": w2p,
            "vp": vp,
            "aux": np.ascontiguousarray(aux),
        })

    global LAST_EXEC_NS, LAST_TRACE_DIR
    import tempfile
    kw = {}
    if TRACE:
        kw = dict(trace=True, tmpdir=tempfile.mkdtemp(prefix="bahdanau_ntff_"))
    res = run_bass_kernel_spmd(nc, in_maps, list(range(NCORES)), **kw)
    LAST_EXEC_NS = res.exec_time_ns
    LAST_TRACE_DIR = kw.get("tmpdir")

    attn = np.empty((B, S, 1), np.float32)
    ctxv = np.empty((B, H), np.float32)
    for c in range(NCORES):
        lo = c * BL
        attn[lo:lo + BL] = np.asarray(res.results[c]["attn_out"]).reshape(BL, S, 1)
        ctxv[lo:lo + BL] = np.asarray(res.results[c]["ctx_out"])
    return attn, ctxv
